# revision 23
# baseline (speedup 1.0000x reference)
"""MoE kernel for Trainium2: M=10 experts (4096->120->84->10), gate-mix to
N=100 task heads (10->120->84->10), B=8192, data-parallel over B on 8 cores.

Per core (B_loc=1024):
  Expert stage: xT resident in SBUF (128KB/part), eW1 streamed once in
    half-expert granules; per expert accumulate L1 over 32 k-chunks, L2,
    then L3 with a host-built block-diagonal W3 so all experts accumulate
    into one PSUM tile E3[(m,f)=100 partitions, B_loc].
  Head stage: gating folded into head layer-1 weights on host
    (W1f[n][(m,f),h] = wg[m,n]*tW1[n][f,h]); head weights streamed in
    25-head chunks (small resident footprint -> ~no stage-transition
    bubble); elementwise work balanced across ACT and DVE.
  Output written as [N, T, B_loc] (contiguous DMA), host transposes.
All matmul operands are float32r (~4e-4 rel err, 1 cycle/column on HW).
"""

import sys
from contextlib import ExitStack

sys.path.insert(0, "/opt/trn_rl_repo")

import numpy as np

import concourse.bacc as bacc
import concourse.mybir as mybir
import concourse.tile as tile
from concourse.bass import ts
from concourse.bass_utils import run_bass_kernel_spmd

f32 = mybir.dt.float32
f32r = mybir.dt.float32r

M, N, F, IN, H1, H2, T = 10, 100, 10, 4096, 120, 84, 10
B, NCORES = 8192, 8
BL = B // NCORES  # 1024 rows per core
KC = IN // 128  # 32 contraction chunks for layer 1
KH = KC // 2  # k-chunks per half-expert weight granule
MF = M * F  # 100 stacked (expert, feature) partitions

LAST_RESULTS = None  # BassKernelResults of the most recent run (for test.py)


def _expert_stage(nc, tc, e3sb, b1s, b2s, e3bs, xt, w1, w2t, w3t):
    Relu = mybir.ActivationFunctionType.Relu
    add = mybir.AluOpType.add
    with ExitStack() as stk:
        pool = lambda name, bufs, **kw: stk.enter_context(
            tc.tile_pool(name=name, bufs=bufs, **kw)
        )
        xp = pool("xp", KC)
        w1p = pool("w1p", 3)
        wA = pool("wA", 1)
        h1p = pool("h1p", 2)
        h2p = pool("h2p", 2)
        psA1 = pool("psA1", 2, space="PSUM")
        psA2 = pool("psA2", 1, space="PSUM")
        psE3 = pool("psE3", 1, space="PSUM")

        # eW1 stream: half-expert granules [128, KH*H1] (k-chunks 0-15 /
        # 16-31 of expert m), one rearranged DMA each on the gpsimd queue.
        w1q = []

        def issue_w1(idx):
            m_, half = divmod(idx, 2)
            wt = w1p.tile([128, KH * H1], f32r, tag="w1t", name=f"w1t_{idx}")
            # gpsimd's engine preamble delays its first DMAs by ~20us; issue
            # the first granules on sync so expert 0 starts immediately
            eng = nc.sync if idx < 2 else nc.gpsimd
            eng.dma_start(out=wt, in_=w1[m_, half])
            w1q.append(wt)

        for idx in range(3):
            issue_w1(idx)

        xk = []
        for k in range(KC):
            xtile = xp.tile([128, BL], f32r, tag="xk", name=f"xk_{k}")
            nc.sync.dma_start(out=xtile, in_=xt[k * 128 : (k + 1) * 128, :])
            xk.append(xtile)
        w2s = wA.tile([H1, M * H2], f32r, tag="w2s")
        w3s = wA.tile([H2, M * MF], f32r, tag="w3s")
        nc.sync.dma_start(out=w2s, in_=w2t)
        nc.sync.dma_start(out=w3s, in_=w3t)

        e3ps = psE3.tile([MF, BL], f32)
        for m in range(M):
            ps1 = psA1.tile([H1, BL], f32, tag="ps1")
            for half in range(2):
                wt = w1q.pop(0)
                for kk in range(KH):
                    k = half * KH + kk
                    for jh in range(2):
                        nc.tensor.matmul(
                            ps1[:, jh * 512 : (jh + 1) * 512],
                            wt[:, ts(kk, H1)],
                            xk[k][:, jh * 512 : (jh + 1) * 512],
                            start=(k == 0),
                            stop=(k == KC - 1),
                        )
                idx = m * 2 + half
                if idx + 3 < 2 * M:
                    issue_w1(idx + 3)
            h1 = h1p.tile([H1, BL], f32r, tag="h1")
            nc.scalar.activation(h1, ps1, Relu, bias=b1s[:, m : m + 1])
            ps2 = psA2.tile([H2, BL], f32, tag="ps2")
            for jh in range(2):
                nc.tensor.matmul(
                    ps2[:, jh * 512 : (jh + 1) * 512],
                    w2s[:, ts(m, H2)],
                    h1[:, jh * 512 : (jh + 1) * 512],
                    start=True,
                    stop=True,
                )
            h2 = h2p.tile([H2, BL], f32r, tag="h2")
            nc.scalar.activation(h2, ps2, Relu, bias=b2s[:, m : m + 1])
            for jh in range(2):
                nc.tensor.matmul(
                    e3ps[:, jh * 512 : (jh + 1) * 512],
                    w3s[:, ts(m, MF)],
                    h2[:, jh * 512 : (jh + 1) * 512],
                    start=(m == 0),
                    stop=(m == M - 1),
                )
        nc.vector.tensor_scalar(e3sb, e3ps, e3bs[:, 0:1], None, add)


def _head_stage(nc, tc, e3sb, tb1s, tb2s, tb3s, w1f, w2f, w3fs, out):
    Relu = mybir.ActivationFunctionType.Relu
    add, amax = mybir.AluOpType.add, mybir.AluOpType.max
    NPC = 20  # heads per streamed weight chunk
    NCH = N // NPC
    with ExitStack() as stk:
        pool = lambda name, bufs, **kw: stk.enter_context(
            tc.tile_pool(name=name, bufs=bufs, **kw)
        )
        w1fp = pool("w1fp", 3)
        w2fp = pool("w2fp", 3)
        t1p = pool("t1p", 2)
        t2p = pool("t2p", 2)
        t3p = pool("t3p", 4)
        psB1 = pool("psB1", 3, space="PSUM")
        psB2 = pool("psB2", 3, space="PSUM")
        psB3 = pool("psB3", 2, space="PSUM")

        w1fcs = {}
        w2fcs = {}

        def issue_head_chunk(c):
            w1fc = w1fp.tile([MF, NPC * H1], f32r, tag="w1fc", name=f"w1fc_{c}")
            nc.sync.dma_start(
                out=w1fc, in_=w1f[:, c * NPC * H1 : (c + 1) * NPC * H1]
            )
            w2fc = w2fp.tile([H1, NPC * H2], f32r, tag="w2fc", name=f"w2fc_{c}")
            nc.gpsimd.dma_start(
                out=w2fc, in_=w2f[:, c * NPC * H2 : (c + 1) * NPC * H2]
            )
            w1fcs[c] = w1fc
            w2fcs[c] = w2fc

        issue_head_chunk(0)
        issue_head_chunk(1)
        for n in range(N):
            c, nn = divmod(n, NPC)
            if nn == 0 and c + 2 < NCH:
                issue_head_chunk(c + 2)
            t1 = t1p.tile([H1, BL], f32r, tag="t1")
            t2 = t2p.tile([H2, BL], f32r, tag="t2")
            for jh in range(2):
                sl = slice(jh * 512, (jh + 1) * 512)
                ps1 = psB1.tile([H1, 512], f32, tag="bps1")
                nc.tensor.matmul(
                    ps1, w1fcs[c][:, ts(nn, H1)], e3sb[:, sl], start=True, stop=True
                )
                nc.scalar.activation(t1[:, sl], ps1, Relu, bias=tb1s[:, n : n + 1])
                ps2 = psB2.tile([H2, 512], f32, tag="bps2")
                nc.tensor.matmul(
                    ps2, w2fcs[c][:, ts(nn, H2)], t1[:, sl], start=True, stop=True
                )
                # balance t2's relu+bias across ACT and DVE
                if jh == 0:
                    nc.scalar.activation(t2[:, sl], ps2, Relu, bias=tb2s[:, n : n + 1])
                else:
                    nc.vector.tensor_scalar(
                        t2[:, sl], ps2, tb2s[:, n : n + 1], 0.0, add, amax
                    )
                ps3 = psB3.tile([T, 512], f32, tag="bps3")
                nc.tensor.matmul(
                    ps3, w3fs[:, ts(n, T)], t2[:, sl], start=True, stop=True
                )
                t3 = t3p.tile([T, 512], f32, tag="t3", name=f"t3_{n}_{jh}")
                nc.vector.tensor_scalar(t3, ps3, tb3s[:, n : n + 1], None, add)
                nc.gpsimd.dma_start(
                    out=out[n, :, jh * 512 : (jh + 1) * 512], in_=t3
                )


def _build_program():
    nc = bacc.Bacc("TRN2", target_bir_lowering=False, debug=False, num_devices=NCORES)

    xt = nc.dram_tensor("xt", [IN, BL], f32r, kind="ExternalInput").ap()
    w1 = nc.dram_tensor("w1", [M, 2, 128, KH * H1], f32r, kind="ExternalInput").ap()
    w2t = nc.dram_tensor("w2t", [H1, M * H2], f32r, kind="ExternalInput").ap()
    w3t = nc.dram_tensor("w3t", [H2, M * MF], f32r, kind="ExternalInput").ap()
    b1t = nc.dram_tensor("b1t", [H1, M], f32, kind="ExternalInput").ap()
    b2t = nc.dram_tensor("b2t", [H2, M], f32, kind="ExternalInput").ap()
    e3b = nc.dram_tensor("e3b", [MF, 1], f32, kind="ExternalInput").ap()
    w1f = nc.dram_tensor("w1f", [MF, N * H1], f32r, kind="ExternalInput").ap()
    tb1t = nc.dram_tensor("tb1t", [H1, N], f32, kind="ExternalInput").ap()
    w2f = nc.dram_tensor("w2f", [H1, N * H2], f32r, kind="ExternalInput").ap()
    tb2t = nc.dram_tensor("tb2t", [H2, N], f32, kind="ExternalInput").ap()
    w3f = nc.dram_tensor("w3f", [H2, N * T], f32r, kind="ExternalInput").ap()
    tb3t = nc.dram_tensor("tb3t", [T, N], f32, kind="ExternalInput").ap()
    out = nc.dram_tensor("out", [N, T, BL], f32, kind="ExternalOutput").ap()

    with tile.TileContext(nc) as tc:
        with tc.tile_pool(name="persist", bufs=1) as persist:
            e3sb = persist.tile([MF, BL], f32r, tag="e3sb")
            b1s = persist.tile([H1, M], f32, tag="b1s")
            b2s = persist.tile([H2, M], f32, tag="b2s")
            e3bs = persist.tile([MF, 1], f32, tag="e3bs")
            tb1s = persist.tile([H1, N], f32, tag="tb1s")
            tb2s = persist.tile([H2, N], f32, tag="tb2s")
            tb3s = persist.tile([T, N], f32, tag="tb3s")
            nc.sync.dma_start(out=b1s, in_=b1t)
            nc.sync.dma_start(out=b2s, in_=b2t)
            nc.sync.dma_start(out=e3bs, in_=e3b)
            nc.sync.dma_start(out=tb1s, in_=tb1t)
            nc.sync.dma_start(out=tb2s, in_=tb2t)
            nc.sync.dma_start(out=tb3s, in_=tb3t)
            w3fs = persist.tile([H2, N * T], f32r, tag="w3fs")
            nc.sync.dma_start(out=w3fs, in_=w3f)

            _expert_stage(nc, tc, e3sb, b1s, b2s, e3bs, xt, w1, w2t, w3t)
            _head_stage(nc, tc, e3sb, tb1s, tb2s, tb3s, w1f, w2f, w3fs, out)

    nc.compile()
    return nc


_PROGRAM = None


def _get_program():
    global _PROGRAM
    if _PROGRAM is None:
        _PROGRAM = _build_program()
    return _PROGRAM


def kernel(x, task, eW1, eb1, eW2, eb2, eW3, eb3,
           gate_w, gate_logits, gate_mask,
           tW1, tb1, tW2, tb2, tW3, tb3):
    global LAST_RESULTS
    f = np.float32
    x = np.asarray(x, f)
    eW1 = np.asarray(eW1, f); eb1 = np.asarray(eb1, f)
    eW2 = np.asarray(eW2, f); eb2 = np.asarray(eb2, f)
    eW3 = np.asarray(eW3, f); eb3 = np.asarray(eb3, f)
    gate_w = np.asarray(gate_w, f)
    gate_logits = np.asarray(gate_logits, f)
    gate_mask_f = np.asarray(gate_mask).astype(f)
    tW1 = np.asarray(tW1, f); tb1 = np.asarray(tb1, f)
    tW2 = np.asarray(tW2, f); tb2 = np.asarray(tb2, f)
    tW3 = np.asarray(tW3, f); tb3 = np.asarray(tb3, f)

    # ---- host-side weight packing ----
    wg = gate_w * gate_mask_f  # [M, N]
    # eW1 granules: [M, 2, 128, KH*H1] with [m, half, p, kk*H1+h] = eW1[m, (half*KH+kk)*128+p, h]
    w1_ = np.ascontiguousarray(
        eW1.reshape(M, 2, KH, 128, H1).transpose(0, 1, 3, 2, 4).reshape(M, 2, 128, KH * H1)
    )
    xT = np.ascontiguousarray(x.T)  # [IN, B]
    w2t_ = np.ascontiguousarray(eW2.transpose(1, 0, 2).reshape(H1, M * H2))
    w3blk = np.zeros((M, H2, MF), f)
    for m in range(M):
        w3blk[m, :, m * F : (m + 1) * F] = eW3[m]
    w3t_ = np.ascontiguousarray(w3blk.transpose(1, 0, 2).reshape(H2, M * MF))
    e3b_ = np.ascontiguousarray(eb3.reshape(MF, 1))
    w1f_ = np.ascontiguousarray(
        np.einsum("mn,nfh->mfnh", wg, tW1).reshape(MF, N * H1)
    )
    w2f_ = np.ascontiguousarray(tW2.transpose(1, 0, 2).reshape(H1, N * H2))
    w3f_ = np.ascontiguousarray(tW3.transpose(1, 0, 2).reshape(H2, N * T))
    b1t_ = np.ascontiguousarray(eb1.T)
    b2t_ = np.ascontiguousarray(eb2.T)
    tb1t_ = np.ascontiguousarray(tb1.T)
    tb2t_ = np.ascontiguousarray(tb2.T)
    tb3t_ = np.ascontiguousarray(tb3.T)

    shared = {
        "w1": w1_, "w2t": w2t_, "w3t": w3t_, "b1t": b1t_, "b2t": b2t_,
        "e3b": e3b_, "w1f": w1f_, "tb1t": tb1t_, "w2f": w2f_,
        "tb2t": tb2t_, "w3f": w3f_, "tb3t": tb3t_,
    }
    in_maps = [
        {"xt": np.ascontiguousarray(xT[:, c * BL : (c + 1) * BL]), **shared}
        for c in range(NCORES)
    ]

    nc = _get_program()
    res = run_bass_kernel_spmd(nc, in_maps, core_ids=list(range(NCORES)))
    LAST_RESULTS = res

    # gather: per-core [N, T, BL] -> [N, T, B] -> [N, B, T]
    out_tb = np.concatenate([res.results[c]["out"] for c in range(NCORES)], axis=2)
    task_out = np.ascontiguousarray(out_tb.transpose(0, 2, 1))

    # logits_loss = sum_m log(sigmoid(gate_logits)) -> [N]   (tiny; host)
    logits_loss = (
        -np.logaddexp(0.0, -gate_logits.astype(np.float64)).sum(axis=0).astype(f)
    )
    return task_out, logits_loss


# revision 25
# speedup vs baseline: 1.1223x; 1.1223x over previous
"""MoE kernel for Trainium2: M=10 experts (4096->120->84->10), gate-mix to
N=100 task heads (10->120->84->10), B=8192, data-parallel over B on 8 cores.

Per core (B_loc=1024):
  Expert stage: xT resident in SBUF (128KB/part), eW1 streamed once in
    half-expert granules; per expert accumulate L1 over 32 k-chunks, L2,
    then L3 with a host-built block-diagonal W3 so all experts accumulate
    into one PSUM tile E3[(m,f)=100 partitions, B_loc].
  Head stage: gating folded into head layer-1 weights on host
    (W1f[n][(m,f),h] = wg[m,n]*tW1[n][f,h]); head weights streamed in
    25-head chunks (small resident footprint -> ~no stage-transition
    bubble); elementwise work balanced across ACT and DVE.
  Output written as [N, T, B_loc] (contiguous DMA), host transposes.
All matmul operands are float32r (~4e-4 rel err, 1 cycle/column on HW).
"""

import sys
from contextlib import ExitStack

sys.path.insert(0, "/opt/trn_rl_repo")

import numpy as np

import concourse.bacc as bacc
import concourse.mybir as mybir
import concourse.tile as tile
from concourse.bass import ts
from concourse.bass_utils import run_bass_kernel_spmd

f32 = mybir.dt.float32
f32r = mybir.dt.float32r

M, N, F, IN, H1, H2, T = 10, 100, 10, 4096, 120, 84, 10
B, NCORES = 8192, 8
BL = B // NCORES  # 1024 rows per core
KC = IN // 128  # 32 contraction chunks for layer 1
KH = KC // 2  # k-chunks per half-expert weight granule
MF = M * F  # 100 stacked (expert, feature) partitions

LAST_RESULTS = None  # BassKernelResults of the most recent run (for test.py)


def _expert_stage(nc, tc, e3sb, b1s, b2s, e3bs, xt, w1, w2t, w3t):
    Relu = mybir.ActivationFunctionType.Relu
    add = mybir.AluOpType.add
    with ExitStack() as stk:
        pool = lambda name, bufs, **kw: stk.enter_context(
            tc.tile_pool(name=name, bufs=bufs, **kw)
        )
        xp = pool("xp", KC)
        w1p = pool("w1p", 3)
        wA = pool("wA", 1)
        h1p = pool("h1p", 2)
        h2p = pool("h2p", 2)
        psA1 = pool("psA1", 2, space="PSUM")
        psA2 = pool("psA2", 1, space="PSUM")
        psE3 = pool("psE3", 1, space="PSUM")

        # eW1 stream: half-expert granules [128, KH*H1] (k-chunks 0-15 /
        # 16-31 of expert m), one rearranged DMA each on the gpsimd queue.
        w1q = []

        def issue_w1(idx):
            m_, half = divmod(idx, 2)
            wt = w1p.tile([128, KH * H1], f32r, tag="w1t", name=f"w1t_{idx}")
            # gpsimd's engine preamble delays its first DMAs by ~20us; issue
            # the first granules on sync so expert 0 starts immediately
            eng = nc.sync if idx < 2 else nc.gpsimd
            eng.dma_start(out=wt, in_=w1[m_, half])
            w1q.append(wt)

        for idx in range(3):
            issue_w1(idx)

        xk = []
        for k in range(KC):
            xtile = xp.tile([128, BL], f32r, tag="xk", name=f"xk_{k}")
            nc.sync.dma_start(out=xtile, in_=xt[k * 128 : (k + 1) * 128, :])
            xk.append(xtile)
        w2s = wA.tile([H1, M * H2], f32r, tag="w2s")
        w3s = wA.tile([H2, M * MF], f32r, tag="w3s")
        nc.sync.dma_start(out=w2s, in_=w2t)
        nc.sync.dma_start(out=w3s, in_=w3t)

        e3ps = psE3.tile([MF, BL], f32)
        for m in range(M):
            ps1 = psA1.tile([H1, BL], f32, tag="ps1")
            for half in range(2):
                wt = w1q.pop(0)
                for kk in range(KH):
                    k = half * KH + kk
                    for jh in range(2):
                        nc.tensor.matmul(
                            ps1[:, jh * 512 : (jh + 1) * 512],
                            wt[:, ts(kk, H1)],
                            xk[k][:, jh * 512 : (jh + 1) * 512],
                            start=(k == 0),
                            stop=(k == KC - 1),
                        )
                idx = m * 2 + half
                if idx + 3 < 2 * M:
                    issue_w1(idx + 3)
            h1 = h1p.tile([H1, BL], f32r, tag="h1")
            nc.scalar.activation(h1, ps1, Relu, bias=b1s[:, m : m + 1])
            ps2 = psA2.tile([H2, BL], f32, tag="ps2")
            for jh in range(2):
                nc.tensor.matmul(
                    ps2[:, jh * 512 : (jh + 1) * 512],
                    w2s[:, ts(m, H2)],
                    h1[:, jh * 512 : (jh + 1) * 512],
                    start=True,
                    stop=True,
                )
            h2 = h2p.tile([H2, BL], f32r, tag="h2")
            nc.scalar.activation(h2, ps2, Relu, bias=b2s[:, m : m + 1])
            for jh in range(2):
                nc.tensor.matmul(
                    e3ps[:, jh * 512 : (jh + 1) * 512],
                    w3s[:, ts(m, MF)],
                    h2[:, jh * 512 : (jh + 1) * 512],
                    start=(m == 0),
                    stop=(m == M - 1),
                )
        nc.vector.tensor_scalar(e3sb, e3ps, e3bs[:, 0:1], None, add)


def _head_stage(nc, tc, e3sb, tb1s, tb2s, tb3s, w1f, w2f, w3fs, out):
    Relu = mybir.ActivationFunctionType.Relu
    add, amax = mybir.AluOpType.add, mybir.AluOpType.max
    # variable chunking: small first chunk so head 0 starts right after the
    # expert stage; bigger steady-state chunks
    CHUNKS = [(0, 8), (8, 31), (39, 31), (70, 30)]
    NCH = len(CHUNKS)
    with ExitStack() as stk:
        pool = lambda name, bufs, **kw: stk.enter_context(
            tc.tile_pool(name=name, bufs=bufs, **kw)
        )
        w1fp = pool("w1fp", 3)
        w2fp = pool("w2fp", 3)
        t1p = pool("t1p", 2)
        t2p = pool("t2p", 2)
        t3p = pool("t3p", 4)
        psB1 = pool("psB1", 3, space="PSUM")
        psB2 = pool("psB2", 3, space="PSUM")
        psB3 = pool("psB3", 2, space="PSUM")

        w1fcs = {}
        w2fcs = {}

        def issue_head_chunk(c):
            c0, cl = CHUNKS[c]
            w1fc = w1fp.tile([MF, cl * H1], f32r, tag="w1fc", name=f"w1fc_{c}")
            nc.sync.dma_start(
                out=w1fc, in_=w1f[:, c0 * H1 : (c0 + cl) * H1]
            )
            w2fc = w2fp.tile([H1, cl * H2], f32r, tag="w2fc", name=f"w2fc_{c}")
            nc.scalar.dma_start(
                out=w2fc, in_=w2f[:, c0 * H2 : (c0 + cl) * H2]
            )
            w1fcs[c] = w1fc
            w2fcs[c] = w2fc

        issue_head_chunk(0)
        issue_head_chunk(1)
        n2c = {}
        for ci, (c0, cl) in enumerate(CHUNKS):
            for nn_ in range(cl):
                n2c[c0 + nn_] = (ci, nn_)
        for n in range(N):
            c, nn = n2c[n]
            if nn == 0 and c + 2 < NCH:
                issue_head_chunk(c + 2)
            t1 = t1p.tile([H1, BL], f32r, tag="t1")
            t2 = t2p.tile([H2, BL], f32r, tag="t2")
            for jh in range(2):
                sl = slice(jh * 512, (jh + 1) * 512)
                ps1 = psB1.tile([H1, 512], f32, tag="bps1")
                nc.tensor.matmul(
                    ps1, w1fcs[c][:, ts(nn, H1)], e3sb[:, sl], start=True, stop=True
                )
                nc.scalar.activation(t1[:, sl], ps1, Relu, bias=tb1s[:, n : n + 1])
                ps2 = psB2.tile([H2, 512], f32, tag="bps2")
                nc.tensor.matmul(
                    ps2, w2fcs[c][:, ts(nn, H2)], t1[:, sl], start=True, stop=True
                )
                # balance t2's relu+bias across ACT and DVE
                if jh == 0:
                    nc.scalar.activation(t2[:, sl], ps2, Relu, bias=tb2s[:, n : n + 1])
                else:
                    nc.vector.tensor_scalar(
                        t2[:, sl], ps2, tb2s[:, n : n + 1], 0.0, add, amax
                    )
                ps3 = psB3.tile([T, 512], f32, tag="bps3")
                nc.tensor.matmul(
                    ps3, w3fs[:, ts(n, T)], t2[:, sl], start=True, stop=True
                )
                t3 = t3p.tile([T, 512], f32, tag="t3", name=f"t3_{n}_{jh}")
                nc.vector.tensor_scalar(t3, ps3, tb3s[:, n : n + 1], None, add)
                nc.gpsimd.dma_start(
                    out=out[n, :, jh * 512 : (jh + 1) * 512], in_=t3
                )


def _build_program():
    nc = bacc.Bacc("TRN2", target_bir_lowering=False, debug=False, num_devices=NCORES)

    xt = nc.dram_tensor("xt", [IN, BL], f32r, kind="ExternalInput").ap()
    w1 = nc.dram_tensor("w1", [M, 2, 128, KH * H1], f32r, kind="ExternalInput").ap()
    w2t = nc.dram_tensor("w2t", [H1, M * H2], f32r, kind="ExternalInput").ap()
    w3t = nc.dram_tensor("w3t", [H2, M * MF], f32r, kind="ExternalInput").ap()
    b1t = nc.dram_tensor("b1t", [H1, M], f32, kind="ExternalInput").ap()
    b2t = nc.dram_tensor("b2t", [H2, M], f32, kind="ExternalInput").ap()
    e3b = nc.dram_tensor("e3b", [MF, 1], f32, kind="ExternalInput").ap()
    w1f = nc.dram_tensor("w1f", [MF, N * H1], f32r, kind="ExternalInput").ap()
    tb1t = nc.dram_tensor("tb1t", [H1, N], f32, kind="ExternalInput").ap()
    w2f = nc.dram_tensor("w2f", [H1, N * H2], f32r, kind="ExternalInput").ap()
    tb2t = nc.dram_tensor("tb2t", [H2, N], f32, kind="ExternalInput").ap()
    w3f = nc.dram_tensor("w3f", [H2, N * T], f32r, kind="ExternalInput").ap()
    tb3t = nc.dram_tensor("tb3t", [T, N], f32, kind="ExternalInput").ap()
    out = nc.dram_tensor("out", [N, T, BL], f32, kind="ExternalOutput").ap()

    with tile.TileContext(nc) as tc:
        with tc.tile_pool(name="persist", bufs=1) as persist:
            e3sb = persist.tile([MF, BL], f32r, tag="e3sb")
            b1s = persist.tile([H1, M], f32, tag="b1s")
            b2s = persist.tile([H2, M], f32, tag="b2s")
            e3bs = persist.tile([MF, 1], f32, tag="e3bs")
            tb1s = persist.tile([H1, N], f32, tag="tb1s")
            tb2s = persist.tile([H2, N], f32, tag="tb2s")
            tb3s = persist.tile([T, N], f32, tag="tb3s")
            nc.sync.dma_start(out=b1s, in_=b1t)
            nc.sync.dma_start(out=b2s, in_=b2t)
            nc.sync.dma_start(out=e3bs, in_=e3b)
            nc.sync.dma_start(out=tb1s, in_=tb1t)
            nc.sync.dma_start(out=tb2s, in_=tb2t)
            nc.sync.dma_start(out=tb3s, in_=tb3t)
            w3fs = persist.tile([H2, N * T], f32r, tag="w3fs")
            nc.sync.dma_start(out=w3fs, in_=w3f)

            _expert_stage(nc, tc, e3sb, b1s, b2s, e3bs, xt, w1, w2t, w3t)
            _head_stage(nc, tc, e3sb, tb1s, tb2s, tb3s, w1f, w2f, w3fs, out)

    nc.compile()
    return nc


_PROGRAM = None


def _get_program():
    global _PROGRAM
    if _PROGRAM is None:
        _PROGRAM = _build_program()
    return _PROGRAM


def kernel(x, task, eW1, eb1, eW2, eb2, eW3, eb3,
           gate_w, gate_logits, gate_mask,
           tW1, tb1, tW2, tb2, tW3, tb3):
    global LAST_RESULTS
    f = np.float32
    x = np.asarray(x, f)
    eW1 = np.asarray(eW1, f); eb1 = np.asarray(eb1, f)
    eW2 = np.asarray(eW2, f); eb2 = np.asarray(eb2, f)
    eW3 = np.asarray(eW3, f); eb3 = np.asarray(eb3, f)
    gate_w = np.asarray(gate_w, f)
    gate_logits = np.asarray(gate_logits, f)
    gate_mask_f = np.asarray(gate_mask).astype(f)
    tW1 = np.asarray(tW1, f); tb1 = np.asarray(tb1, f)
    tW2 = np.asarray(tW2, f); tb2 = np.asarray(tb2, f)
    tW3 = np.asarray(tW3, f); tb3 = np.asarray(tb3, f)

    # ---- host-side weight packing ----
    wg = gate_w * gate_mask_f  # [M, N]
    # eW1 granules: [M, 2, 128, KH*H1] with [m, half, p, kk*H1+h] = eW1[m, (half*KH+kk)*128+p, h]
    w1_ = np.ascontiguousarray(
        eW1.reshape(M, 2, KH, 128, H1).transpose(0, 1, 3, 2, 4).reshape(M, 2, 128, KH * H1)
    )
    xT = np.ascontiguousarray(x.T)  # [IN, B]
    w2t_ = np.ascontiguousarray(eW2.transpose(1, 0, 2).reshape(H1, M * H2))
    w3blk = np.zeros((M, H2, MF), f)
    for m in range(M):
        w3blk[m, :, m * F : (m + 1) * F] = eW3[m]
    w3t_ = np.ascontiguousarray(w3blk.transpose(1, 0, 2).reshape(H2, M * MF))
    e3b_ = np.ascontiguousarray(eb3.reshape(MF, 1))
    w1f_ = np.ascontiguousarray(
        np.einsum("mn,nfh->mfnh", wg, tW1).reshape(MF, N * H1)
    )
    w2f_ = np.ascontiguousarray(tW2.transpose(1, 0, 2).reshape(H1, N * H2))
    w3f_ = np.ascontiguousarray(tW3.transpose(1, 0, 2).reshape(H2, N * T))
    b1t_ = np.ascontiguousarray(eb1.T)
    b2t_ = np.ascontiguousarray(eb2.T)
    tb1t_ = np.ascontiguousarray(tb1.T)
    tb2t_ = np.ascontiguousarray(tb2.T)
    tb3t_ = np.ascontiguousarray(tb3.T)

    shared = {
        "w1": w1_, "w2t": w2t_, "w3t": w3t_, "b1t": b1t_, "b2t": b2t_,
        "e3b": e3b_, "w1f": w1f_, "tb1t": tb1t_, "w2f": w2f_,
        "tb2t": tb2t_, "w3f": w3f_, "tb3t": tb3t_,
    }
    in_maps = [
        {"xt": np.ascontiguousarray(xT[:, c * BL : (c + 1) * BL]), **shared}
        for c in range(NCORES)
    ]

    nc = _get_program()
    res = run_bass_kernel_spmd(nc, in_maps, core_ids=list(range(NCORES)))
    LAST_RESULTS = res

    # gather: per-core [N, T, BL] -> [N, T, B] -> [N, B, T]
    out_tb = np.concatenate([res.results[c]["out"] for c in range(NCORES)], axis=2)
    task_out = np.ascontiguousarray(out_tb.transpose(0, 2, 1))

    # logits_loss = sum_m log(sigmoid(gate_logits)) -> [N]   (tiny; host)
    logits_loss = (
        -np.logaddexp(0.0, -gate_logits.astype(np.float64)).sum(axis=0).astype(f)
    )
    return task_out, logits_loss
